# revision 25
# baseline (speedup 1.0000x reference)
"""Multi-head attention Bass/Tile kernel for TRN2, 8-core SPMD.

Sharding: core c handles batch b = c//2 and head-group g = c%2 (6 of 12 heads).
Each core computes its 6 heads end-to-end plus a partial output projection
(over its 384 of 768 ctx dims); the host sums the two partials per batch.

Layout strategy (all matmuls bf16 streams into fp32 PSUM):
  - bf16 for hs / Wq / Wk / Wv / WoT / qT / kT / ctxT / v_aug / exp tiles
    (halves input DMA; PE runs at full rate either way).
  - mini-warmup matmuls bridge the pre-DMA idle so the PE HAM clock-gate
    is at 8/8 when real work arrives.
  - phase 1 is e-outer: all 8 q/k PSUM tiles of an m-chunk live at once,
    consuming each hs chunk as its DMA lands (DMA-lockstep start ~2us in).
  - exp split per score group: A-tile on ScalarE (native Exp), B-tile on
    VectorE via a bf16 Schraudolph bit-trick bitcast_bf16(int16(A*x + B))
    (~3.4% element error, washes out through the softmax average).
  - softmax normalize, pair-batched: ScalarE+DVE evacuate both heads into
    one [96,1024] tile, ONE reciprocal_approx_fast over both denominators,
    ONE DRAM-bounce partition-broadcast DMA, multiplies on gpsimd (kept on
    a single ucode library; DVE for the final pair to shorten the tail).
  - output projection fused lag-1 into phase 2: two e-chunks per head-pair
    through the phase-2 PSUM ring; the final s-block runs f-outer across
    3 ring slots so only its last matmuls wait on the last normalize.
"""

from contextlib import ExitStack

import numpy as np

import concourse.bass as bass
import concourse.tile as tile
from concourse import bacc, mybir
from concourse._compat import with_exitstack


F32 = mybir.dt.float32
BF16 = mybir.dt.bfloat16
I32 = mybir.dt.int32
AF = mybir.ActivationFunctionType
ALU = mybir.AluOpType

B, E, S, H, D = 4, 768, 2048, 12, 64
VW0 = 96
NH = 6          # heads per core
HD = NH * D     # 384 head-dims per core
NE = E // 128   # 6 e-chunks
NM = HD // 128  # 3 m-chunks (2 heads each)
NT = S // 128   # 16 t-tiles
SBW = 512       # s-block width
NS = S // SBW   # 4 s-blocks

# Schraudolph exp: bitcast_f32(int32(SCH_A * x + SCH_B)) ~ exp(x), max rel
# err ~3.0%; errors average out through the softmax-weighted sum over ~10^2
# participating tokens.
SCH_A = float(2.0**23 / np.log(2.0))
SCH_B = 1064986716.0

# exp tiles routed to the DVE instead of ScalarE: B-tile of these g values
# (per (s, p) there are 8 groups x {A-tile, B-tile}).
DVE_B_GROUPS = (0, 1, 2, 3, 4, 5, 6, 7)
GPSIMD_MUL = True   # normalize multiplies on gpsimd (else vector)


@with_exitstack
def mha_tile(ctx: ExitStack, tc, hs, wq, wk, wv, bq, bk, bv, woT, bo2, outT):
    nc = tc.nc

    persist = ctx.enter_context(tc.tile_pool(name="persist", bufs=1))

    # --- warmup: keep the PE busy while input DMA streams in, so the HAM
    # clock-gate is at 8/8 when real matmuls start ---
    warm_sb = persist.tile([128, 512], F32R, name="warm_sb")
    nc.vector.memset(warm_sb[:].bitcast(F32), 0.0)
    with tc.tile_pool(name="warmps", bufs=1, space="PSUM") as warmps:
        wps = warmps.tile([128, 512], F32, name="wps")
        for _ in range(WARMUP_MMS):
            nc.tensor.matmul(
                wps[:], warm_sb[:, 0:128], warm_sb[:], start=True, stop=True,
                skip_group_check=True,
            )

    # --- weights / constants (emit hs + wq first: first consumers) ---
    wq_sb = [persist.tile([128, HD], BF16, name=f"wq{e}") for e in range(NE)]
    wk_sb = [persist.tile([128, HD], BF16, name=f"wk{e}") for e in range(NE)]
    wv_sb = [persist.tile([128, HD], BF16, name=f"wv{e}") for e in range(NE)]
    woT_sb = [persist.tile([128, E], BF16, name=f"wo{f}") for f in range(NM)]

    hsp = ctx.enter_context(tc.tile_pool(name="hsp", bufs=1))
    hs_sb = [hsp.tile([128, S], BF16, name=f"hs{e}") for e in range(NE)]
    for e in range(NE):
        sl = slice(128 * e, 128 * (e + 1))
        nc.sync.dma_start(wq_sb[e][:], wq[sl, :])
        nc.sync.dma_start(wk_sb[e][:], wk[sl, :])
        nc.sync.dma_start(hs_sb[e][:], hs[sl, :])
    for e in range(NE):
        sl = slice(128 * e, 128 * (e + 1))
        nc.sync.dma_start(wv_sb[e][:], wv[sl, :])
    for f in range(NM):
        nc.sync.dma_start(woT_sb[f][:], woT[128 * f : 128 * (f + 1), :])

    bq_sb = persist.tile([128, NM], F32, name="bq")
    bk_sb = persist.tile([128, NM], F32, name="bk")
    nc.sync.dma_start(bq_sb[:], bq.rearrange("(m p) -> p m", p=128))
    nc.sync.dma_start(bk_sb[:], bk.rearrange("(m p) -> p m", p=128))
    bv_bc = persist.tile([128, HD], F32, name="bv")
    nc.sync.dma_start(
        bv_bc[:], bass.AP(tensor=bv.tensor, offset=bv.offset, ap=[[0, 128], [1, HD]])
    )
    bo_sb = persist.tile([128, NE], F32, name="bo")
    nc.sync.dma_start(bo_sb[:], bo2.rearrange("(m p) -> p m", p=128))

    # v_aug[t]: [128 tokens, head, 96]: col 0 = ones (softmax denominator on
    # PSUM partition 0), cols 32..95 = v-dims. cols 1..31 are never read
    # downstream (ctx rows 1..31 are junk) so they stay uninitialized.
    VW = 96
    v_aug = [persist.tile([128, NH, VW], F32R, name=f"vaug{t}") for t in range(NT)]
    for t in range(NT):
        nc.vector.memset(v_aug[t][:, :, 0:1].bitcast(F32), 1.0)

    qT_sb = [persist.tile([128, S], BF16, name=f"qT{m}") for m in range(NM)]
    kT_sb = [persist.tile([128, S], BF16, name=f"kT{m}") for m in range(NM)]
    ctxT_sb = [persist.tile([128, S], BF16, name=f"ctxT{m}") for m in range(NM)]

    # --- phase 1: QKV projections ---
    with tc.tile_pool(name="ps1", bufs=4, space="PSUM") as ps1:
        for m in range(NM):
            msl = slice(128 * m, 128 * (m + 1))
            for s in range(NS):
                ssl = slice(SBW * s, SBW * (s + 1))
                qp = ps1.tile([128, SBW], F32, tag="qk", bufs=6)
                for e in range(NE):
                    nc.tensor.matmul(
                        qp[:], wq_sb[e][:, msl], hs_sb[e][:, ssl],
                        start=(e == 0), stop=(e == NE - 1),
                    )
                nc.vector.tensor_scalar_add(
                    out=qT_sb[m][:, ssl], in0=qp[:], scalar1=bq_sb[:, m : m + 1]
                )
                kp = ps1.tile([128, SBW], F32, tag="qk", bufs=6)
                for e in range(NE):
                    nc.tensor.matmul(
                        kp[:], wk_sb[e][:, msl], hs_sb[e][:, ssl],
                        start=(e == 0), stop=(e == NE - 1),
                    )
                nc.scalar.activation(
                    kT_sb[m][:, ssl], kp[:], AF.Identity, bias=bk_sb[:, m : m + 1]
                )

        for t in range(NT):
            tsl = slice(128 * t, 128 * (t + 1))
            vp = ps1.tile([128, HD], F32, tag="v", bufs=2)
            for e in range(NE):
                nc.tensor.matmul(
                    vp[:], hs_sb[e][:, tsl], wv_sb[e][:, :],
                    start=(e == 0), stop=(e == NE - 1),
                )
            nc.vector.tensor_add(
                out=v_aug[t][:, :, 32 : 32 + D].bitcast(F32),
                in0=vp[:].rearrange("p (h d) -> p h d", h=NH),
                in1=bv_bc[:].rearrange("p (h d) -> p h d", h=NH),
            )

    # --- phase 2: attention (sc -> exp -> ctx pipelined), with the output
    # projection for s-block s-1 fused after head-pair p=0 of s-block s ---
    with (
        tc.tile_pool(name="expp", bufs=6) as expp,
        tc.tile_pool(name="smp", bufs=3) as smp,
        tc.tile_pool(name="pssc", bufs=3, space="PSUM") as pssc,
        tc.tile_pool(name="psctx", bufs=1, space="PSUM") as psctx,
        tc.tile_pool(name="outp", bufs=4) as outp,
    ):
        GW = 2 * SBW  # score-group width: 2 t-tiles per exp instruction
        recd = nc.dram_tensor("recd", [NS * NM, GW], F32, kind="Internal")

        def normalize_pair(cpA, cpB, p, ssl, slot, last=False):
            # pair-batched normalize: evacuate both heads' den+ctx into one
            # [VW, 1024] tile (A half on ScalarE, B half on DVE), ONE
            # fast-approx reciprocal over both denominators, ONE DRAM-bounce
            # partition broadcast, multiplies on gpsimd (DVE for the final
            # pair to shorten the tail chain)
            cu = smp.tile([VW, GW], F32, tag="cu")
            nc.scalar.copy(cu[:, 0:SBW], cpA[0:VW, :])
            nc.vector.tensor_copy(cu[:, SBW:GW], cpB[0:VW, :])
            rec = smp.tile([1, GW], F32, tag="rec")
            nc.vector.reciprocal_approx_fast(out=rec[0:1, :], in_=cu[0:1, :])
            if last:
                # PE-side broadcast: ones[1,96].T @ rec_bf[1,512] fills the
                # just-freed ctx PSUM banks with the reciprocal rows -- skips
                # the two serialized DMA hops right before the final matmuls
                rec_bf = smp.tile([1, GW], BF16, tag="recbf")
                nc.vector.tensor_copy(rec_bf[:], rec[0:1, :])
                bcpA = psctx.tile([128, SBW], F32, tag="ctxA", name="bcpA")
                bcpB = psctx.tile([128, SBW], F32, tag="ctxB", name="bcpB")
                nc.tensor.matmul(
                    bcpA[0:VW, :], ones96[0:1, :], rec_bf[0:1, 0:SBW],
                    start=True, stop=True,
                )
                nc.tensor.matmul(
                    bcpB[0:VW, :], ones96[0:1, :], rec_bf[0:1, SBW:GW],
                    start=True, stop=True,
                )
                for h in (2 * p, 2 * p + 1):
                    m, o = h // 2, D * (h % 2)
                    cs = SBW * (h % 2)
                    bcp = bcpA if h % 2 == 0 else bcpB
                    for q in range(2):
                        nc.vector.tensor_mul(
                            out=ctxT_sb[m][o + 32 * q : o + 32 * (q + 1), ssl],
                            in0=cu[32 + 32 * q : 64 + 32 * q, cs : cs + SBW],
                            in1=bcp[32 + 32 * q : 64 + 32 * q, :],
                        )
                return
            row = recd[slot : slot + 1, :]
            nc.sync.dma_start(row, rec[0:1, :])
            bcs = smp.tile([VW, GW], F32, tag="bcs")
            nc.sync.dma_start(
                bcs[:],
                bass.AP(tensor=row.tensor, offset=row.offset,
                        ap=[[0, VW], [1, GW]]),
            )
            eng = nc.gpsimd if GPSIMD_MUL else nc.vector
            for h in (2 * p, 2 * p + 1):
                m, o = h // 2, D * (h % 2)
                cs = SBW * (h % 2)
                for q in range(2):  # two 32-partition chunks (alignment rules)
                    eng.tensor_mul(
                        out=ctxT_sb[m][o + 32 * q : o + 32 * (q + 1), ssl],
                        in0=cu[32 + 32 * q : 64 + 32 * q, cs : cs + SBW],
                        in1=bcs[32 + 32 * q : 64 + 32 * q, cs : cs + SBW],
                    )

        def out_proj(s, ets=range(NE)):
            # both e-chunks share ONE [128, GW] ring slot (halves the slots a
            # new pair's sc allocations can block on at the ring wrap)
            ssl = slice(SBW * s, SBW * (s + 1))
            ets = list(ets)
            op = pssc.tile([128, GW], F32, tag="sc")
            for i, et in enumerate(ets):
                esl = slice(128 * et, 128 * (et + 1))
                csl = slice(SBW * i, SBW * (i + 1))
                for f in range(NM):
                    nc.tensor.matmul(
                        op[:, csl], woT_sb[f][:, esl], ctxT_sb[f][:, ssl],
                        start=(f == 0), stop=(f == NM - 1),
                    )
            for i, et in enumerate(ets):
                esl = slice(128 * et, 128 * (et + 1))
                csl = slice(SBW * i, SBW * (i + 1))
                ob = outp.tile([128, SBW], F32, tag="ob")
                if et % 2 == 0:
                    nc.scalar.activation(
                        ob[:], op[:, csl], AF.Identity, bias=bo_sb[:, et : et + 1]
                    )
                else:
                    nc.vector.tensor_scalar_add(
                        out=ob[:], in0=op[:, csl], scalar1=bo_sb[:, et : et + 1]
                    )
                nc.sync.dma_start(outT[esl, ssl], ob[:])

        for s in range(NS):
            ssl = slice(SBW * s, SBW * (s + 1))
            for p in range(NM):  # head pair p = heads (2p, 2p+1)
                if p > 0 and s > 0:
                    # out-proj chunk at pair START: independent PE fill that
                    # absorbs the ring-wrap wait, and its slot's evacs clear
                    # mid-pair instead of blocking the next pair's sc allocs
                    out_proj(s - 1, range(2 * p, 2 * p + 2))
                kTh = kT_sb[p]
                qTh = qT_sb[p]
                cpA = psctx.tile([128, SBW], F32, tag="ctxA")
                cpB = psctx.tile([128, SBW], F32, tag="ctxB")
                pend = None
                for g in range(NT // 2):  # groups of 2 t-tiles
                    scA = pssc.tile([128, GW], F32, tag="sc")
                    scB = pssc.tile([128, GW], F32, tag="sc")
                    for j in range(2):
                        t = 2 * g + j
                        tsl = slice(128 * t, 128 * (t + 1))
                        nc.tensor.matmul(
                            scA[:, SBW * j : SBW * (j + 1)],
                            kTh[0:D, tsl], qTh[0:D, ssl],
                            start=True, stop=True,
                        )
                        nc.tensor.matmul(
                            scB[:, SBW * j : SBW * (j + 1)],
                            kTh[D:128, tsl], qTh[D:128, ssl],
                            start=True, stop=True,
                        )
                    exA = expp.tile([128, GW], F32R, tag="exp")
                    nc.scalar.activation(exA[:], scA[:], AF.Exp)
                    exB = expp.tile([128, GW], F32R, tag="exp")
                    if g in DVE_B_GROUPS:
                        nc.vector.tensor_scalar(
                            out=exB[:].bitcast(I32), in0=scB[:],
                            scalar1=SCH_A, scalar2=SCH_B,
                            op0=ALU.mult, op1=ALU.add,
                        )
                    else:
                        nc.scalar.activation(exB[:], scB[:], AF.Exp)
                    if pend is not None:
                        pA, pB, pg = pend
                        for j in range(2):
                            t = 2 * pg + j
                            st = t == 0
                            nc.tensor.matmul(
                                cpA[0:VW, :], v_aug[t][:, 2 * p, :],
                                pA[:, SBW * j : SBW * (j + 1)],
                                start=st, stop=False,
                            )
                            nc.tensor.matmul(
                                cpB[0:VW, :], v_aug[t][:, 2 * p + 1, :],
                                pB[:, SBW * j : SBW * (j + 1)],
                                start=st, stop=False,
                            )
                    pend = (exA, exB, g)
                pA, pB, pg = pend
                for j in range(2):
                    t = 2 * pg + j
                    sp = t == NT - 1
                    nc.tensor.matmul(
                        cpA[0:VW, :], v_aug[t][:, 2 * p, :],
                        pA[:, SBW * j : SBW * (j + 1)],
                        start=False, stop=sp,
                    )
                    nc.tensor.matmul(
                        cpB[0:VW, :], v_aug[t][:, 2 * p + 1, :],
                        pB[:, SBW * j : SBW * (j + 1)],
                        start=False, stop=sp,
                    )
                if s > 0 and p == 0:
                    # p=0's chunk stays at pair END: at pair start the
                    # previous s-block's last normalize is still in flight
                    out_proj(s - 1, range(0, 2))
                last = s == NS - 1 and p == NM - 1
                normalize_pair(cpA, cpB, p, ssl, s * NM + p, last)

        # final s-block out-projection, f-outer: et-chunk pairs share the 3
        # [128, GW] ring slots; f=0/1 matmuls run while the last pair's
        # normalize is still in flight, only f=2 waits on it
        ssl = slice(SBW * (NS - 1), SBW * NS)
        slots = [pssc.tile([128, GW], F32, tag="sc", name=f"fop{k}") for k in range(NM)]
        for f in range(NM):
            for et in range(NE):
                esl = slice(128 * et, 128 * (et + 1))
                csl = slice(SBW * (et % 2), SBW * (et % 2 + 1))
                nc.tensor.matmul(
                    slots[et // 2][:, csl], woT_sb[f][:, esl], ctxT_sb[f][:, ssl],
                    start=(f == 0), stop=(f == NM - 1),
                )
        for et in range(NE):
            esl = slice(128 * et, 128 * (et + 1))
            csl = slice(SBW * (et % 2), SBW * (et % 2 + 1))
            ob = outp.tile([128, SBW], F32, tag="ob")
            nc.scalar.activation(
                ob[:], slots[et // 2][:, csl], AF.Identity,
                bias=bo_sb[:, et : et + 1],
            )
            nc.sync.dma_start(outT[esl, ssl], ob[:])


def build_nc():
    nc = bacc.Bacc("TRN2", target_bir_lowering=False, debug=False)
    hs = nc.dram_tensor("hs", [E, S], BF16, kind="ExternalInput")
    wq = nc.dram_tensor("wq", [E, HD], BF16, kind="ExternalInput")
    wk = nc.dram_tensor("wk", [E, HD], BF16, kind="ExternalInput")
    wv = nc.dram_tensor("wv", [E, HD], BF16, kind="ExternalInput")
    bq = nc.dram_tensor("bq", [HD], F32, kind="ExternalInput")
    bk = nc.dram_tensor("bk", [HD], F32, kind="ExternalInput")
    bv = nc.dram_tensor("bv", [HD], F32, kind="ExternalInput")
    woT = nc.dram_tensor("woT", [HD, E], BF16, kind="ExternalInput")
    bo2 = nc.dram_tensor("bo2", [E], F32, kind="ExternalInput")
    outT = nc.dram_tensor("outT", [E, S], F32, kind="ExternalOutput")

    with tile.TileContext(nc) as tc:
        mha_tile(
            tc,
            hs[:, :], wq[:, :], wk[:, :], wv[:, :],
            bq[:], bk[:], bv[:],
            woT[:, :], bo2[:], outT[:, :],
        )
    nc.compile()
    return nc


def make_core_inputs(inputs: dict) -> list[dict]:
    """Full inputs -> per-core input maps (core c: batch c//2, head-group c%2)."""
    import ml_dtypes

    bf16 = ml_dtypes.bfloat16
    hsf = np.asarray(inputs["hidden_state"], dtype=np.float32)
    Wq = np.asarray(inputs["Wq"], dtype=np.float32)
    Wk = np.asarray(inputs["Wk"], dtype=np.float32)
    Wv = np.asarray(inputs["Wv"], dtype=np.float32)
    Wo = np.asarray(inputs["Wo"], dtype=np.float32)
    bq = np.asarray(inputs["bq"], dtype=np.float32)
    bk = np.asarray(inputs["bk"], dtype=np.float32)
    bv = np.asarray(inputs["bv"], dtype=np.float32)
    bo = np.asarray(inputs["bo"], dtype=np.float32)

    maps = []
    for c in range(8):
        b, g = c // 2, c % 2
        hsl = slice(NH * g, NH * (g + 1))
        fsl = slice(HD * g, HD * (g + 1))
        maps.append(
            {
                "hs": np.ascontiguousarray(hsf[b].astype(bf16)),
                "wq": np.ascontiguousarray(
                    Wq[hsl].transpose(1, 0, 2).reshape(E, HD).astype(bf16)
                ),
                "wk": np.ascontiguousarray(
                    Wk[hsl].transpose(1, 0, 2).reshape(E, HD).astype(bf16)
                ),
                "wv": np.ascontiguousarray(
                    Wv[hsl].transpose(1, 0, 2).reshape(E, HD).astype(bf16)
                ),
                "bq": np.ascontiguousarray(bq[hsl].reshape(HD)),
                "bk": np.ascontiguousarray(bk[hsl].reshape(HD)),
                "bv": np.ascontiguousarray(bv[hsl].reshape(HD)),
                "woT": np.ascontiguousarray(Wo[:, fsl].T.astype(bf16)),
                "bo2": np.ascontiguousarray(bo / 2.0),
            }
        )
    return maps


def combine_outputs(core_outs: list) -> np.ndarray:
    """Per-core outT partials -> full [B, E, S] output."""
    return np.stack(
        [core_outs[2 * b]["outT"] + core_outs[2 * b + 1]["outT"] for b in range(B)]
    ).astype(np.float32)


from concourse.bass_utils import run_bass_kernel_spmd

N_CORES = 8
_NC_CACHE = None


def _get_nc():
    global _NC_CACHE
    if _NC_CACHE is None:
        _NC_CACHE = build_nc()
    return _NC_CACHE


def kernel(**inputs) -> np.ndarray:
    """Full-input entry point: shard across 8 cores, run, unshard."""
    maps = make_core_inputs(inputs)
    nc = _get_nc()
    res = run_bass_kernel_spmd(nc, maps, core_ids=list(range(N_CORES)))
    outs = res.results
    return np.stack(
        [outs[2 * b]["outT"] + outs[2 * b + 1]["outT"] for b in range(B)]
    ).astype(np.float32)


# revision 26
# speedup vs baseline: 1.0157x; 1.0157x over previous
"""Multi-head attention Bass/Tile kernel for TRN2, 8-core SPMD.

Sharding: core c handles batch b = c//2 and head-group g = c%2 (6 of 12 heads).
Each core computes its 6 heads end-to-end plus a partial output projection
(over its 384 of 768 ctx dims); the host sums the two partials per batch.

Layout strategy (all matmuls bf16 streams into fp32 PSUM):
  - bf16 for hs / Wq / Wk / Wv / WoT / qT / kT / ctxT / v_aug / exp tiles
    (halves input DMA; PE runs at full rate either way).
  - mini-warmup matmuls bridge the pre-DMA idle so the PE HAM clock-gate
    is at 8/8 when real work arrives.
  - phase 1 is e-outer: all 8 q/k PSUM tiles of an m-chunk live at once,
    consuming each hs chunk as its DMA lands (DMA-lockstep start ~2us in).
  - exp split per score group: A-tile on ScalarE (native Exp), B-tile on
    VectorE via a bf16 Schraudolph bit-trick bitcast_bf16(int16(A*x + B))
    (~3.4% element error, washes out through the softmax average).
  - softmax normalize, pair-batched: ScalarE+DVE evacuate both heads into
    one [96,1024] tile, ONE reciprocal_approx_fast over both denominators,
    ONE DRAM-bounce partition-broadcast DMA, multiplies on gpsimd (kept on
    a single ucode library; DVE for the final pair to shorten the tail).
  - output projection fused lag-1 into phase 2: two e-chunks per head-pair
    through the phase-2 PSUM ring; the final s-block runs f-outer across
    3 ring slots so only its last matmuls wait on the last normalize.
"""

from contextlib import ExitStack

import numpy as np

import concourse.bass as bass
import concourse.tile as tile
from concourse import bacc, mybir
from concourse._compat import with_exitstack


F32 = mybir.dt.float32
BF16 = mybir.dt.bfloat16
I32 = mybir.dt.int32
AF = mybir.ActivationFunctionType
ALU = mybir.AluOpType

B, E, S, H, D = 4, 768, 2048, 12, 64
VW0 = 96
NH = 6          # heads per core
HD = NH * D     # 384 head-dims per core
NE = E // 128   # 6 e-chunks
NM = HD // 128  # 3 m-chunks (2 heads each)
NT = S // 128   # 16 t-tiles
SBW = 512       # s-block width
NS = S // SBW   # 4 s-blocks

# Schraudolph exp: bitcast_f32(int32(SCH_A * x + SCH_B)) ~ exp(x), max rel
# err ~3.0%; errors average out through the softmax-weighted sum over ~10^2
# participating tokens.
SCH_A = float(2.0**23 / np.log(2.0))
SCH_B = 1064986716.0

# exp tiles routed to the DVE instead of ScalarE: B-tile of these g values
# (per (s, p) there are 8 groups x {A-tile, B-tile}).
DVE_B_GROUPS = (0, 1, 2, 3, 4, 5, 6, 7)
GPSIMD_MUL = True   # normalize multiplies on gpsimd (else vector)


@with_exitstack
def mha_tile(ctx: ExitStack, tc, hs, wq, wk, wv, bq, bk, bv, woT, bo2, outT):
    nc = tc.nc

    persist = ctx.enter_context(tc.tile_pool(name="persist", bufs=1))

    # --- warmup: keep the PE busy while input DMA streams in, so the HAM
    # clock-gate is at 8/8 when real matmuls start ---
    warm_sb = persist.tile([128, 512], F32R, name="warm_sb")
    nc.vector.memset(warm_sb[:].bitcast(F32), 0.0)
    with tc.tile_pool(name="warmps", bufs=1, space="PSUM") as warmps:
        wps = warmps.tile([128, 512], F32, name="wps")
        for _ in range(WARMUP_MMS):
            nc.tensor.matmul(
                wps[:], warm_sb[:, 0:128], warm_sb[:], start=True, stop=True,
                skip_group_check=True,
            )

    # --- weights / constants (emit hs + wq first: first consumers) ---
    wq_sb = [persist.tile([128, HD], BF16, name=f"wq{e}") for e in range(NE)]
    wk_sb = [persist.tile([128, HD], BF16, name=f"wk{e}") for e in range(NE)]
    wv_sb = [persist.tile([128, HD], BF16, name=f"wv{e}") for e in range(NE)]
    woT_sb = [persist.tile([128, E], BF16, name=f"wo{f}") for f in range(NM)]

    hsp = ctx.enter_context(tc.tile_pool(name="hsp", bufs=1))
    hs_sb = [hsp.tile([128, S], BF16, name=f"hs{e}") for e in range(NE)]
    for e in range(NE):
        sl = slice(128 * e, 128 * (e + 1))
        nc.sync.dma_start(wq_sb[e][:], wq[sl, :])
        nc.sync.dma_start(wk_sb[e][:], wk[sl, :])
        nc.sync.dma_start(hs_sb[e][:], hs[sl, :])
    for e in range(NE):
        sl = slice(128 * e, 128 * (e + 1))
        nc.sync.dma_start(wv_sb[e][:], wv[sl, :])
    for f in range(NM):
        nc.sync.dma_start(woT_sb[f][:], woT[128 * f : 128 * (f + 1), :])

    bq_sb = persist.tile([128, NM], F32, name="bq")
    bk_sb = persist.tile([128, NM], F32, name="bk")
    nc.sync.dma_start(bq_sb[:], bq.rearrange("(m p) -> p m", p=128))
    nc.sync.dma_start(bk_sb[:], bk.rearrange("(m p) -> p m", p=128))
    bv_bc = persist.tile([128, HD], F32, name="bv")
    nc.sync.dma_start(
        bv_bc[:], bass.AP(tensor=bv.tensor, offset=bv.offset, ap=[[0, 128], [1, HD]])
    )
    bo_sb = persist.tile([128, NE], F32, name="bo")
    nc.sync.dma_start(bo_sb[:], bo2.rearrange("(m p) -> p m", p=128))

    # v_aug[t]: [128 tokens, head, 96]: col 0 = ones (softmax denominator on
    # PSUM partition 0), cols 32..95 = v-dims. cols 1..31 are never read
    # downstream (ctx rows 1..31 are junk) so they stay uninitialized.
    VW = 96
    v_aug = [persist.tile([128, NH, VW], F32R, name=f"vaug{t}") for t in range(NT)]
    for t in range(NT):
        nc.vector.memset(v_aug[t][:, :, 0:1].bitcast(F32), 1.0)

    qT_sb = [persist.tile([128, S], BF16, name=f"qT{m}") for m in range(NM)]
    kT_sb = [persist.tile([128, S], BF16, name=f"kT{m}") for m in range(NM)]
    ctxT_sb = [persist.tile([128, S], BF16, name=f"ctxT{m}") for m in range(NM)]

    # --- phase 1: QKV projections ---
    with tc.tile_pool(name="ps1", bufs=4, space="PSUM") as ps1:
        for m in range(NM):
            msl = slice(128 * m, 128 * (m + 1))
            for s in range(NS):
                ssl = slice(SBW * s, SBW * (s + 1))
                qp = ps1.tile([128, SBW], F32, tag="qk", bufs=6)
                for e in range(NE):
                    nc.tensor.matmul(
                        qp[:], wq_sb[e][:, msl], hs_sb[e][:, ssl],
                        start=(e == 0), stop=(e == NE - 1),
                    )
                nc.vector.tensor_scalar_add(
                    out=qT_sb[m][:, ssl], in0=qp[:], scalar1=bq_sb[:, m : m + 1]
                )
                kp = ps1.tile([128, SBW], F32, tag="qk", bufs=6)
                for e in range(NE):
                    nc.tensor.matmul(
                        kp[:], wk_sb[e][:, msl], hs_sb[e][:, ssl],
                        start=(e == 0), stop=(e == NE - 1),
                    )
                nc.scalar.activation(
                    kT_sb[m][:, ssl], kp[:], AF.Identity, bias=bk_sb[:, m : m + 1]
                )

        for t in range(NT):
            tsl = slice(128 * t, 128 * (t + 1))
            vp = ps1.tile([128, HD], F32, tag="v", bufs=2)
            for e in range(NE):
                nc.tensor.matmul(
                    vp[:], hs_sb[e][:, tsl], wv_sb[e][:, :],
                    start=(e == 0), stop=(e == NE - 1),
                )
            nc.vector.tensor_add(
                out=v_aug[t][:, :, 32 : 32 + D].bitcast(F32),
                in0=vp[:].rearrange("p (h d) -> p h d", h=NH),
                in1=bv_bc[:].rearrange("p (h d) -> p h d", h=NH),
            )

    # --- phase 2: attention (sc -> exp -> ctx pipelined), with the output
    # projection for s-block s-1 fused after head-pair p=0 of s-block s ---
    with (
        tc.tile_pool(name="expp", bufs=6) as expp,
        tc.tile_pool(name="smp", bufs=3) as smp,
        tc.tile_pool(name="pssc", bufs=3, space="PSUM") as pssc,
        tc.tile_pool(name="psctx", bufs=1, space="PSUM") as psctx,
        tc.tile_pool(name="outp", bufs=4) as outp,
    ):
        GW = 2 * SBW  # score-group width: 2 t-tiles per exp instruction
        recd = nc.dram_tensor("recd", [NS * NM, GW], F32, kind="Internal")

        def normalize_pair(cpA, cpB, p, ssl, slot, last=False):
            # pair-batched normalize: evacuate both heads' den+ctx into one
            # [VW, 1024] tile (A half on ScalarE, B half on DVE), ONE
            # fast-approx reciprocal over both denominators, ONE DRAM-bounce
            # partition broadcast, multiplies on gpsimd (DVE for the final
            # pair to shorten the tail chain)
            cu = smp.tile([VW, GW], F32, tag="cu")
            nc.scalar.copy(cu[:, 0:SBW], cpA[0:VW, :])
            nc.vector.tensor_copy(cu[:, SBW:GW], cpB[0:VW, :])
            rec = smp.tile([1, GW], F32, tag="rec")
            nc.vector.reciprocal_approx_fast(out=rec[0:1, :], in_=cu[0:1, :])
            if last:
                # PE-side broadcast: ones[1,96].T @ rec_bf[1,512] fills the
                # just-freed ctx PSUM banks with the reciprocal rows -- skips
                # the two serialized DMA hops right before the final matmuls
                rec_bf = smp.tile([1, GW], BF16, tag="recbf")
                nc.vector.tensor_copy(rec_bf[:], rec[0:1, :])
                bcpA = psctx.tile([128, SBW], F32, tag="ctxA", name="bcpA")
                bcpB = psctx.tile([128, SBW], F32, tag="ctxB", name="bcpB")
                nc.tensor.matmul(
                    bcpA[0:VW, :], ones96[0:1, :], rec_bf[0:1, 0:SBW],
                    start=True, stop=True,
                )
                nc.tensor.matmul(
                    bcpB[0:VW, :], ones96[0:1, :], rec_bf[0:1, SBW:GW],
                    start=True, stop=True,
                )
                for h in (2 * p, 2 * p + 1):
                    m, o = h // 2, D * (h % 2)
                    cs = SBW * (h % 2)
                    bcp = bcpA if h % 2 == 0 else bcpB
                    for q in range(2):
                        nc.vector.tensor_mul(
                            out=ctxT_sb[m][o + 32 * q : o + 32 * (q + 1), ssl],
                            in0=cu[32 + 32 * q : 64 + 32 * q, cs : cs + SBW],
                            in1=bcp[32 + 32 * q : 64 + 32 * q, :],
                        )
                return
            row = recd[slot : slot + 1, :]
            nc.sync.dma_start(row, rec[0:1, :])
            bcs = smp.tile([VW, GW], F32, tag="bcs")
            nc.sync.dma_start(
                bcs[:],
                bass.AP(tensor=row.tensor, offset=row.offset,
                        ap=[[0, VW], [1, GW]]),
            )
            eng = nc.gpsimd if GPSIMD_MUL else nc.vector
            for h in (2 * p, 2 * p + 1):
                m, o = h // 2, D * (h % 2)
                cs = SBW * (h % 2)
                for q in range(2):  # two 32-partition chunks (alignment rules)
                    eng.tensor_mul(
                        out=ctxT_sb[m][o + 32 * q : o + 32 * (q + 1), ssl],
                        in0=cu[32 + 32 * q : 64 + 32 * q, cs : cs + SBW],
                        in1=bcs[32 + 32 * q : 64 + 32 * q, cs : cs + SBW],
                    )

        def out_proj(s, ets=range(NE)):
            # both e-chunks share ONE [128, GW] ring slot (halves the slots a
            # new pair's sc allocations can block on at the ring wrap)
            ssl = slice(SBW * s, SBW * (s + 1))
            ets = list(ets)
            op = pssc.tile([128, GW], F32, tag="sc")
            for i, et in enumerate(ets):
                esl = slice(128 * et, 128 * (et + 1))
                csl = slice(SBW * i, SBW * (i + 1))
                for f in range(NM):
                    nc.tensor.matmul(
                        op[:, csl], woT_sb[f][:, esl], ctxT_sb[f][:, ssl],
                        start=(f == 0), stop=(f == NM - 1),
                    )
            for i, et in enumerate(ets):
                esl = slice(128 * et, 128 * (et + 1))
                csl = slice(SBW * i, SBW * (i + 1))
                ob = outp.tile([128, SBW], F32, tag="ob")
                if et % 2 == 0:
                    nc.scalar.activation(
                        ob[:], op[:, csl], AF.Identity, bias=bo_sb[:, et : et + 1]
                    )
                else:
                    nc.vector.tensor_scalar_add(
                        out=ob[:], in0=op[:, csl], scalar1=bo_sb[:, et : et + 1]
                    )
                nc.sync.dma_start(outT[esl, ssl], ob[:])

        for s in range(NS):
            ssl = slice(SBW * s, SBW * (s + 1))
            for p in range(NM):  # head pair p = heads (2p, 2p+1)
                kTh = kT_sb[p]
                qTh = qT_sb[p]
                cpA = psctx.tile([128, SBW], F32, tag="ctxA")
                cpB = psctx.tile([128, SBW], F32, tag="ctxB")
                pend = None
                for g in range(NT // 2):  # groups of 2 t-tiles
                    scA = pssc.tile([128, GW], F32, tag="sc")
                    scB = pssc.tile([128, GW], F32, tag="sc")
                    for j in range(2):
                        t = 2 * g + j
                        tsl = slice(128 * t, 128 * (t + 1))
                        nc.tensor.matmul(
                            scA[:, SBW * j : SBW * (j + 1)],
                            kTh[0:D, tsl], qTh[0:D, ssl],
                            start=True, stop=True,
                        )
                        nc.tensor.matmul(
                            scB[:, SBW * j : SBW * (j + 1)],
                            kTh[D:128, tsl], qTh[D:128, ssl],
                            start=True, stop=True,
                        )
                    exA = expp.tile([128, GW], F32R, tag="exp")
                    nc.scalar.activation(exA[:], scA[:], AF.Exp)
                    exB = expp.tile([128, GW], F32R, tag="exp")
                    if g in DVE_B_GROUPS:
                        nc.vector.tensor_scalar(
                            out=exB[:].bitcast(I32), in0=scB[:],
                            scalar1=SCH_A, scalar2=SCH_B,
                            op0=ALU.mult, op1=ALU.add,
                        )
                    else:
                        nc.scalar.activation(exB[:], scB[:], AF.Exp)
                    if pend is not None:
                        pA, pB, pg = pend
                        for j in range(2):
                            t = 2 * pg + j
                            st = t == 0
                            nc.tensor.matmul(
                                cpA[0:VW, :], v_aug[t][:, 2 * p, :],
                                pA[:, SBW * j : SBW * (j + 1)],
                                start=st, stop=False,
                            )
                            nc.tensor.matmul(
                                cpB[0:VW, :], v_aug[t][:, 2 * p + 1, :],
                                pB[:, SBW * j : SBW * (j + 1)],
                                start=st, stop=False,
                            )
                    pend = (exA, exB, g)
                pA, pB, pg = pend
                for j in range(2):
                    t = 2 * pg + j
                    sp = t == NT - 1
                    nc.tensor.matmul(
                        cpA[0:VW, :], v_aug[t][:, 2 * p, :],
                        pA[:, SBW * j : SBW * (j + 1)],
                        start=False, stop=sp,
                    )
                    nc.tensor.matmul(
                        cpB[0:VW, :], v_aug[t][:, 2 * p + 1, :],
                        pB[:, SBW * j : SBW * (j + 1)],
                        start=False, stop=sp,
                    )
                if s > 0:
                    # lag-1 fusion, 2 e-chunks per pair: spreads the evac
                    # load and PE fill evenly across pair boundaries
                    out_proj(s - 1, range(2 * p, 2 * p + 2))
                last = s == NS - 1 and p == NM - 1
                normalize_pair(cpA, cpB, p, ssl, s * NM + p, last)

        # final s-block out-projection, f-outer: et-chunk pairs share the 3
        # [128, GW] ring slots; f=0/1 matmuls run while the last pair's
        # normalize is still in flight, only f=2 waits on it
        ssl = slice(SBW * (NS - 1), SBW * NS)
        slots = [pssc.tile([128, GW], F32, tag="sc", name=f"fop{k}") for k in range(NM)]
        for f in range(NM):
            for et in range(NE):
                esl = slice(128 * et, 128 * (et + 1))
                csl = slice(SBW * (et % 2), SBW * (et % 2 + 1))
                nc.tensor.matmul(
                    slots[et // 2][:, csl], woT_sb[f][:, esl], ctxT_sb[f][:, ssl],
                    start=(f == 0), stop=(f == NM - 1),
                )
        for et in range(NE):
            esl = slice(128 * et, 128 * (et + 1))
            csl = slice(SBW * (et % 2), SBW * (et % 2 + 1))
            ob = outp.tile([128, SBW], F32, tag="ob")
            nc.scalar.activation(
                ob[:], slots[et // 2][:, csl], AF.Identity,
                bias=bo_sb[:, et : et + 1],
            )
            nc.sync.dma_start(outT[esl, ssl], ob[:])


def build_nc():
    nc = bacc.Bacc("TRN2", target_bir_lowering=False, debug=False)
    hs = nc.dram_tensor("hs", [E, S], BF16, kind="ExternalInput")
    wq = nc.dram_tensor("wq", [E, HD], BF16, kind="ExternalInput")
    wk = nc.dram_tensor("wk", [E, HD], BF16, kind="ExternalInput")
    wv = nc.dram_tensor("wv", [E, HD], BF16, kind="ExternalInput")
    bq = nc.dram_tensor("bq", [HD], F32, kind="ExternalInput")
    bk = nc.dram_tensor("bk", [HD], F32, kind="ExternalInput")
    bv = nc.dram_tensor("bv", [HD], F32, kind="ExternalInput")
    woT = nc.dram_tensor("woT", [HD, E], BF16, kind="ExternalInput")
    bo2 = nc.dram_tensor("bo2", [E], F32, kind="ExternalInput")
    outT = nc.dram_tensor("outT", [E, S], F32, kind="ExternalOutput")

    with tile.TileContext(nc) as tc:
        mha_tile(
            tc,
            hs[:, :], wq[:, :], wk[:, :], wv[:, :],
            bq[:], bk[:], bv[:],
            woT[:, :], bo2[:], outT[:, :],
        )
    nc.compile()
    return nc


def make_core_inputs(inputs: dict) -> list[dict]:
    """Full inputs -> per-core input maps (core c: batch c//2, head-group c%2)."""
    import ml_dtypes

    bf16 = ml_dtypes.bfloat16
    hsf = np.asarray(inputs["hidden_state"], dtype=np.float32)
    Wq = np.asarray(inputs["Wq"], dtype=np.float32)
    Wk = np.asarray(inputs["Wk"], dtype=np.float32)
    Wv = np.asarray(inputs["Wv"], dtype=np.float32)
    Wo = np.asarray(inputs["Wo"], dtype=np.float32)
    bq = np.asarray(inputs["bq"], dtype=np.float32)
    bk = np.asarray(inputs["bk"], dtype=np.float32)
    bv = np.asarray(inputs["bv"], dtype=np.float32)
    bo = np.asarray(inputs["bo"], dtype=np.float32)

    maps = []
    for c in range(8):
        b, g = c // 2, c % 2
        hsl = slice(NH * g, NH * (g + 1))
        fsl = slice(HD * g, HD * (g + 1))
        maps.append(
            {
                "hs": np.ascontiguousarray(hsf[b].astype(bf16)),
                "wq": np.ascontiguousarray(
                    Wq[hsl].transpose(1, 0, 2).reshape(E, HD).astype(bf16)
                ),
                "wk": np.ascontiguousarray(
                    Wk[hsl].transpose(1, 0, 2).reshape(E, HD).astype(bf16)
                ),
                "wv": np.ascontiguousarray(
                    Wv[hsl].transpose(1, 0, 2).reshape(E, HD).astype(bf16)
                ),
                "bq": np.ascontiguousarray(bq[hsl].reshape(HD)),
                "bk": np.ascontiguousarray(bk[hsl].reshape(HD)),
                "bv": np.ascontiguousarray(bv[hsl].reshape(HD)),
                "woT": np.ascontiguousarray(Wo[:, fsl].T.astype(bf16)),
                "bo2": np.ascontiguousarray(bo / 2.0),
            }
        )
    return maps


def combine_outputs(core_outs: list) -> np.ndarray:
    """Per-core outT partials -> full [B, E, S] output."""
    return np.stack(
        [core_outs[2 * b]["outT"] + core_outs[2 * b + 1]["outT"] for b in range(B)]
    ).astype(np.float32)


from concourse.bass_utils import run_bass_kernel_spmd

N_CORES = 8
_NC_CACHE = None


def _get_nc():
    global _NC_CACHE
    if _NC_CACHE is None:
        _NC_CACHE = build_nc()
    return _NC_CACHE


def kernel(**inputs) -> np.ndarray:
    """Full-input entry point: shard across 8 cores, run, unshard."""
    maps = make_core_inputs(inputs)
    nc = _get_nc()
    res = run_bass_kernel_spmd(nc, maps, core_ids=list(range(N_CORES)))
    outs = res.results
    return np.stack(
        [outs[2 * b]["outT"] + outs[2 * b + 1]["outT"] for b in range(B)]
    ).astype(np.float32)
